# revision 2
# baseline (speedup 1.0000x reference)
"""Covariance pooling kernel for Trainium2 (8 NeuronCores, data-parallel over batch).

y[b] = (1/M) * (x[b] - mean(x[b])) @ (x[b] - mean(x[b]))^T  with x[b] [C=128, M=4096].

The kernel is HBM-read bound: 16.78 MB fp32 per core. Strategy:
  - one SWDGE cast DMA per batch (fp32 HBM -> fp8_e4m3 SBUF): 16 KB reads per
    descriptor row stream at ~355 GB/s/core and are contention-immune, unlike
    finer splits; the last batch is split 4x so the PE tail after the final
    packet is one chunk-group, not a whole batch
  - all 8 fp8 batches stay resident in SBUF (32 KB/partition) so every DMA is
    enqueued up front with no buffer-reuse waits anywhere in the stream
  - PE transposes chunk pairs (fp8, mandatory element-step-2 PSUM writes at
    even offsets), one DVE/ACT copy interleaves a pair into an SBUF slot, and
    one DoubleRowSwInterleave matmul (0.5 cyc/col) accumulates both chunks:
    PE work is ~29 us, fully hidden under the ~47 us DMA stream
  - DoubleRowSwInterleave reads stationary columns reversed, so PSUM holds
    P@G (rows flipped) and column 128 holds P@s from the ones column; the
    rank-1 centering uses s_row (straight transpose) and s_row reversed via a
    matmul against the anti-identity J; the host un-flips rows for free
"""

import numpy as np

import ml_dtypes
import concourse.bass as bass
import concourse.tile as tile
from concourse import bacc, mybir
from concourse.bass_utils import run_bass_kernel_spmd

N_CORES = 8
B_FULL = 64
B_CORE = B_FULL // N_CORES  # 8 batches per core
C = 128
M = 4096  # 64*64 spatial
PAIRS = M // 256  # 16 chunk pairs per batch
NSLOT = 8  # SBUF pair-slot ring
F32 = mybir.dt.float32
BF16 = mybir.dt.bfloat16
FP8 = mybir.dt.float8e4
COPY = mybir.ActivationFunctionType.Copy
DRSW = mybir.MatmulPerfMode.DoubleRowSwInterleave

_CACHE: dict = {}


def _build_program() -> bass.Bass:
    nc = bacc.Bacc()
    x = nc.declare_dram_parameter("x", [B_CORE, C, M], F32, isOutput=False)
    ident8 = nc.declare_dram_parameter("ident8", [C, C], FP8, isOutput=False)
    identb = nc.declare_dram_parameter("identb", [C, C], BF16, isOutput=False)
    jrev = nc.declare_dram_parameter("jrev", [C, C], BF16, isOutput=False)
    y = nc.declare_dram_parameter("y", [B_CORE, C, C], F32, isOutput=True)

    with tile.TileContext(nc) as tc:
        with (
            tc.tile_pool(name="singles", bufs=1) as singles,
            tc.tile_pool(name="yout", bufs=3) as yout_pool,
            tc.tile_pool(name="small", bufs=4) as small_pool,
            tc.tile_pool(name="tp", bufs=3, space="PSUM") as tp_pool,
            tc.tile_pool(name="gram", bufs=3, space="PSUM") as gram_pool,
            tc.tile_pool(name="srow", bufs=1, space="PSUM") as srow_pool,
            tc.tile_pool(name="srev", bufs=1, space="PSUM") as srev_pool,
        ):
            # the whole input, fp8, resident: DMAs enqueue back-to-back with
            # no reuse hazards; 16 KB-per-row descriptors for peak throughput
            xb = singles.tile([C, B_CORE, M], FP8)
            for b in range(B_CORE):
                if b < B_CORE - 1:
                    nc.gpsimd.dma_start(xb[:, b], x[b][:, :])
                else:
                    step = M // 4  # finer completion grain to shrink the tail
                    for h in range(4):
                        nc.gpsimd.dma_start(
                            xb[:, b, h * step : (h + 1) * step],
                            x[b][:, h * step : (h + 1) * step],
                        )

            identity8 = singles.tile([C, C], FP8)
            nc.sync.dma_start(identity8, ident8[:, :])
            identityb = singles.tile([C, C], BF16)
            nc.sync.dma_start(identityb, identb[:, :])
            J = singles.tile([C, C], BF16)
            nc.sync.dma_start(J, jrev[:, :])

            # pair slots: col c pair t at byte 2c+t; col 128 = ones column
            # (feeds row sums through the gram matmul), col 129 = zero pad
            xt = singles.tile([C, NSLOT, 130, 2], FP8)
            nc.vector.memset(xt[:, :, 128, :], 1.0)
            nc.vector.memset(xt[:, :, 129, :], 0.0)

            # PE warm-up absorbs the identity-DMA wait before data arrives
            warm = tp_pool.tile([C, 2, C, 2], FP8, tag="tp")
            nc.tensor.transpose(warm[:, 0, :, 0], identity8, identity8)

            for b in range(B_CORE):
                gram = gram_pool.tile([C, 130], F32)
                for g in range(PAIRS):
                    tp = tp_pool.tile([C, 2, C, 2], FP8, tag="tp")
                    for j in range(2):
                        k = 2 * g + j
                        nc.tensor.transpose(
                            tp[:, j, :, 0],
                            xb[:, b, k * 128 : (k + 1) * 128],
                            identity8,
                        )
                    s = g % NSLOT
                    dst = xt[:, s, 0:128, :].rearrange("p c t -> p t c")
                    if g % 2 == 0:
                        nc.vector.tensor_copy(dst, tp[:, :, :, 0])
                    else:
                        nc.scalar.activation(dst, tp[:, :, :, 0], COPY)
                    nc.tensor.matmul(
                        gram,
                        xt[:, s, 0:128, :],
                        xt[:, s, 0:130, :].rearrange("p c t -> p t c"),
                        start=(g == 0),
                        stop=False,
                        perf_mode=DRSW,
                    )

                # epilogue: PSUM holds [P@G | P@s]; add -(P@s) s^T / M, scale
                s_col = small_pool.tile([C, 1], BF16)
                nc.vector.tensor_copy(s_col, gram[:, 128:129])
                s_row_ps = srow_pool.tile([1, C], BF16)
                nc.tensor.transpose(s_row_ps, s_col, identityb)
                s_rev_ps = srev_pool.tile([1, C], F32)
                nc.tensor.matmul(s_rev_ps, s_col, J, start=True, stop=True)
                srow = small_pool.tile([1, C], BF16)
                nc.vector.tensor_copy(srow, s_row_ps)
                srow_neg = small_pool.tile([1, C], BF16)
                nc.vector.tensor_scalar_mul(srow_neg, s_rev_ps, -1.0 / M)
                nc.tensor.matmul(gram[:, 0:128], srow, srow_neg, start=False, stop=True)

                y_tile = yout_pool.tile([C, C], F32)
                nc.vector.tensor_scalar_mul(y_tile, gram[:, 0:128], 1.0 / M)
                nc.sync.dma_start(y[b], y_tile)

    nc.compile()
    return nc


def _get_program() -> bass.Bass:
    if "nc" not in _CACHE:
        _CACHE["nc"] = _build_program()
    return _CACHE["nc"]


def _run(x: np.ndarray, **spmd_kwargs):
    x = np.ascontiguousarray(np.asarray(x), dtype=np.float32)
    assert x.shape == (B_FULL, C, 64, 64), x.shape
    xf = x.reshape(B_FULL, C, M)
    shards = np.split(xf, N_CORES, axis=0)
    ident8 = np.eye(C, dtype=ml_dtypes.float8_e4m3)
    identb = np.eye(C, dtype=ml_dtypes.bfloat16)
    jrev = np.eye(C, dtype=ml_dtypes.bfloat16)[::-1].copy()
    in_maps = [
        {"x": s, "ident8": ident8, "identb": identb, "jrev": jrev} for s in shards
    ]
    nc = _get_program()
    res = run_bass_kernel_spmd(nc, in_maps, list(range(N_CORES)), **spmd_kwargs)
    # row un-flip: the DoubleRowSwInterleave gram leaves P@y in DRAM
    out = np.concatenate(
        [res.results[i]["y"][:, ::-1, :] for i in range(N_CORES)], axis=0
    )
    return np.ascontiguousarray(out), res


def kernel(x: np.ndarray) -> np.ndarray:
    out, _ = _run(x)
    return out
